# revision 16
# baseline (speedup 1.0000x reference)
"""CapsuleLayer kernel for 8x TRN2 NeuronCores (Bass/Tile, SPMD).

Math (reference collapses because routing logits b stay zero):
  s[b,o,h,w]  = sum_ic conv2d(u[b,ic], W[ic], SAME) + sum_ic bias[ic]
              = conv2d(u[b] as 64ch, Wcat[256,64,5,5]) + bias_sum      (one conv)
  r(h,w)      = 1 / (8 * nvalid(h,w))              (input-independent constant)
  p           = r * s ; sq[oc] = sum_od p^2
  v           = p * sq/((1+sq)*sqrt(sq+1e-9))
  out[b,oc,od,h,w] = v

Sharding: 8 cores = (batch b in 0..4) x (OC half in 0..2). Each core runs a
64->128-channel 5x5 conv over one 128x128 image + squash, fully on-chip.

Conv as 13 matmuls per 512-pixel tile (K=128, M=128, N=512, fp16):
  - 10 row-paired: K packs 64ch x 2 kernel rows (dy in {0,1} / {2,3}) via a
    second image copy shifted one row (partitions 64-127), x 5 kw taps.
  - 3 col-paired: kernel row dy=4, K packs 64ch x 2 kw taps via a second
    tensor whose partitions 64-127 hold the image shifted one column.
  (25 taps * 64ch = 1600 = 12.5 * 128, so 13 matmuls is the K=128 floor.)

Squash: per-tile cross-partition sq-reduce via one PE matmul with a 0/1 mask
whose output partition is m = oc*32 + j (oc = capsule quadrant, j = tile index
within the group), so the G broadcast is a single Vector STREAM_SHUFFLE
(mask=[j]*32 replicates partition j within each 32-partition quadrant) instead
of a PE matmul. The scalar chain uses only {square, ln, exp} (single forced
ACT table set):  G = r * exp(0.5*ln(sq+1e-9) - ln(1+sq));  v = s * G.

Scheduling: reduce lagged one conv tile behind its ACT square (add_dep_helper),
a few PE warmup matmuls during the input DMA to pre-arm the HAM clock gate,
inputs split across both HWDGE rings (+ SWDGE for the r constants), f16 output.
"""

import numpy as np


def _ensure_path():
    try:
        import concourse.bass  # noqa: F401
    except ImportError:
        import sys

        for p in ("/opt/trn_rl_repo", "/root/.axon_site/_ro/trn_rl_repo"):
            if p not in sys.path:
                sys.path.insert(0, p)
        import concourse.bass  # noqa: F401


B, IC, CIN, H, W = 4, 4, 16, 128, 128
KS, OC, OD = 5, 8, 32
CC = IC * CIN            # 64 contraction channels
NOCH = 128               # out channels per core (4 capsules x 32 dims)
PADH, PADW = H + 5, W + 4   # 133 x 132 (extra pad row for the shifted copy)
PADH2 = 128              # upad2 rows = pad rows 4..131 (dy=4 taps only)
NPIX = H * W
TPX = 512                # pixels per tile (4 rows)
NT = NPIX // TPX         # 32 tiles
GROUPS = [8, 6, 6, 4, 3, 2, 3]   # tiles per squash group; small groups are
NGR = len(GROUPS)                # spread so tail chains don't stack on Vector
NKT = 13                 # matmuls per conv tile: 10 row-paired + 3 col-paired
NWARM = 8

_BUILD_CACHE = {}


def _build_program():
    """Build the SPMD Bass program (same for every core)."""
    if "nc" in _BUILD_CACHE:
        return _BUILD_CACHE["nc"]
    _ensure_path()
    import concourse.bacc as bacc
    import concourse.mybir as mybir
    import concourse.tile as tile
    from concourse.tile import add_dep_helper

    f32 = mybir.dt.float32
    f16 = mybir.dt.float16
    AF = mybir.ActivationFunctionType
    OP = mybir.AluOpType

    # Square/Ln/Exp/Identity all live in the 'natural_log_exp_and_others' ACT
    # table set, but the default set picker uses a different home set per
    # function (2 table reloads x 1.3us per group). Restrict the choice so a
    # single table load covers the whole kernel.
    if not getattr(bacc, "_capsule_act_patch", False):
        _orig_tables = bacc.get_activation_tables

        def _one_set_tables(arch):
            t = _orig_tables(arch)
            keep = "natural_log_exp_and_others"
            if keep in t:
                t = {k: (v if k == keep else set()) for k, v in t.items()}
            return t

        bacc.get_activation_tables = _one_set_tables
        bacc._capsule_act_patch = True

    nc = bacc.Bacc("TRN2", target_bir_lowering=False, debug=False, num_devices=8)

    upad_d = nc.dram_tensor("upad", [128, PADH * PADW], f16, kind="ExternalInput").ap()
    upad2_d = nc.dram_tensor("upad2", [128, PADH2 * PADW], f16, kind="ExternalInput").ap()
    wt_d = nc.dram_tensor("wt", [128, NKT * NOCH], f16, kind="ExternalInput").ap()
    # mred carries the f32 bias bit-pattern in 2 trailing f16 columns: a
    # standalone [128,1] f32 DMA degenerates to 128 4-byte packets that each
    # burn a round-robin slot on the ring (~3.4us of queue time).
    mred_d = nc.dram_tensor("mred", [128, 8 * NOCH + 2], f16, kind="ExternalInput").ap()
    sel_d = nc.dram_tensor("sel", [128, GROUPS[-1] * NOCH], f16, kind="ExternalInput").ap()
    rr_d = nc.dram_tensor("rr", [128, NGR * TPX], f32, kind="ExternalInput").ap()
    out_d = nc.dram_tensor("out", [128, NPIX], f16, kind="ExternalOutput").ap()

    with tile.TileContext(nc) as tc:
        with (
            tc.tile_pool(name="const", bufs=1) as cpool,
            tc.tile_pool(name="sg", bufs=3) as sgpool,
            tc.tile_pool(name="sq", bufs=4) as sqpool,
            tc.tile_pool(name="chain", bufs=2) as chpool,
            tc.tile_pool(name="gv", bufs=3) as gvpool,
            tc.tile_pool(name="sh", bufs=3) as shpool,
            tc.tile_pool(name="vout", bufs=3) as vpool,
            tc.tile_pool(name="cps", bufs=4, space="PSUM") as cps,
            tc.tile_pool(name="gps", bufs=2, space="PSUM") as gps,
            tc.tile_pool(name="bps", bufs=2, space="PSUM") as bps,
        ):
            # DMA order matters, and so do chunk sizes: the SDMA engines
            # round-robin between queues at PACKET granularity, so early
            # chunks are kept small and strictly in need-order; bulk data is
            # deferred. The r constants ride SWDGE, dep-anchored behind tile 2
            # so they stay out of the startup window entirely.
            wt_t = cpool.tile([128, NKT * NOCH], f16)
            nc.sync.dma_start(wt_t[:], wt_d[:])
            upad2_t = cpool.tile([128, PADH2 * PADW], f16)
            upad23 = upad2_t[:].rearrange("p (y x) -> p y x", x=PADW)
            u2src3 = upad2_d.rearrange("p (y x) -> p y x", x=PADW)
            nc.sync.dma_start(upad23[:, 0:16, :], u2src3[:, 0:16, :])
            mred_t = cpool.tile([128, 8 * NOCH + 2], f16)
            nc.sync.dma_start(mred_t[:], mred_d[:])
            bias_t = mred_t[:, 8 * NOCH : 8 * NOCH + 2].bitcast(f32)
            for r0, r1 in [(16, 48), (48, 96), (96, PADH2)]:
                nc.sync.dma_start(upad23[:, r0:r1, :], u2src3[:, r0:r1, :])
            sel_t = cpool.tile([128, GROUPS[-1] * NOCH], f16)
            nc.sync.dma_start(sel_t[:], sel_d[:])

            upad_t = cpool.tile([128, PADH * PADW], f16)
            upad3 = upad_t[:].rearrange("p (y x) -> p y x", x=PADW)
            usrc3 = upad_d.rearrange("p (y x) -> p y x", x=PADW)
            for r0, r1 in [(0, 7), (7, 15), (15, 47), (47, 90), (90, PADH)]:
                nc.scalar.dma_start(upad3[:, r0:r1, :], usrc3[:, r0:r1, :])

            rr_t = cpool.tile([128, NGR * TPX], f32)
            r4_t = cpool.tile([128, NGR * TPX], f32)
            eps_t = cpool.tile([128, 1], f32)
            nc.vector.memset(eps_t[:], 1e-9)

            # PE warmup: junk matmuls while the input DMAs land, so the HAM
            # clock gate is ramping toward 8/8 (2.4 GHz) when real conv work
            # starts. Operands alias the const-0.0 tile (written in the Bass
            # preamble BEFORE the Tile entry barrier) bitcast to f16.
            warm_ps = bps.tile([128, TPX], f32, tag="warm", name="warmps")
            c16 = nc.const_aps.aps[(f32, 0.0)].bitcast(f16)
            wrhs = c16[:, 0:1].to_broadcast((128, TPX))
            wlhs = c16[:, 0:1].to_broadcast((128, 128))
            for k in range(NWARM):
                nc.tensor.matmul(
                    warm_ps[:],
                    wlhs,
                    wrhs,
                    start=(k == 0),
                    stop=(k == NWARM - 1),
                )

            first_tile = [0] * NGR      # first global tile index per group
            acc = 0
            for gi, gsz in enumerate(GROUPS):
                first_tile[gi] = acc
                acc += gsz

            s_tiles = {}
            g_tiles = {}
            gp_tiles = {}

            def emit_conv_tile(gi, j):
                t = first_tile[gi] + j
                y0 = 4 * t
                cp = cps.tile([128, TPX], f32, tag="convps")
                ti = 0
                last_mm = None
                # 10 row-paired matmuls (dy in {0,1} and {2,3})
                for dyp in range(2):
                    for kw in range(KS):
                        rhs = upad3[:, y0 + 2 * dyp : y0 + 2 * dyp + 4, kw : kw + W]
                        last_mm = nc.tensor.matmul(
                            cp[:],
                            wt_t[:, ti * NOCH : (ti + 1) * NOCH],
                            rhs,
                            start=(ti == 0),
                            stop=False,
                        )
                        ti += 1
                # 3 col-paired matmuls (dy=4, kw pairs {0,1},{2,3},{4,-})
                for kwp in range(3):
                    rhs = upad23[:, y0 : y0 + 4, 2 * kwp : 2 * kwp + W]
                    last_mm = nc.tensor.matmul(
                        cp[:],
                        wt_t[:, ti * NOCH : (ti + 1) * NOCH],
                        rhs,
                        start=False,
                        stop=(ti == NKT - 1),
                    )
                    ti += 1
                # Square(cp + bias) straight from PSUM (fused bias, faster
                # PSUM read, and independent of the add below)
                sq = sqpool.tile([128, TPX], f16, tag="sqt")
                nc.scalar.activation(sq[:], cp[:], AF.Square, bias=bias_t[:, 0:1])
                s_sl = s_tiles[gi][:, j * TPX : (j + 1) * TPX]
                nc.scalar.add(s_sl, cp[:], bias_t[:, 0:1])
                return sq, last_mm

            def emit_red(gi, j, sq, anchor):
                gsz = GROUPS[gi]
                red_mm = nc.tensor.matmul(
                    gp_tiles[gi][:],
                    mred_t[:, j * NOCH : (j + 1) * NOCH],
                    sq[:],
                    start=(j == 0),
                    stop=(j == gsz - 1),
                )
                if anchor is not None:
                    # keep the reduce AFTER the just-emitted conv tile in the
                    # PE stream so its ACT square input is long done (the
                    # scheduler would otherwise hoist it next to its producer
                    # and stall the PE ~0.5-1us per tile)
                    add_dep_helper(
                        red_mm.ins, anchor.ins, sync=True, reason="lag red"
                    )
                return red_mm

            def emit_chain(gi):
                gp = gp_tiles[gi]
                r4 = r4_t[:, gi * TPX : (gi + 1) * TPX]
                r1 = rr_t[:, gi * TPX : (gi + 1) * TPX]
                sqv = chpool.tile([128, TPX], f32, tag="sqv")
                nc.vector.tensor_mul(sqv[:], gp[:], r4)
                ln_a = chpool.tile([128, TPX], f32, tag="ln_a")
                nc.scalar.activation(ln_a[:], sqv[:], AF.Ln, bias=eps_t[:, 0:1])
                ln_b = chpool.tile([128, TPX], f32, tag="ln_b")
                nc.scalar.activation(ln_b[:], sqv[:], AF.Ln, bias=1.0)
                dd = chpool.tile([128, TPX], f32, tag="dd")
                nc.vector.scalar_tensor_tensor(
                    dd[:], ln_a[:], 0.5, ln_b[:], OP.mult, OP.subtract
                )
                ee = chpool.tile([128, TPX], f32, tag="ee")
                nc.scalar.activation(ee[:], dd[:], AF.Exp)
                gt_ = gvpool.tile([128, TPX], f16, tag="g16")
                nc.vector.tensor_mul(gt_[:], ee[:], r1)
                g_tiles[gi] = gt_

            def emit_chain_last(gi):
                # Half-split chain for the tail group: the Scalar/Vector stages
                # of the two halves ping-pong, cutting the serial latency that
                # sits after the final conv matmul.
                gp = gp_tiles[gi]
                HP = TPX // 2
                gt_ = gvpool.tile([128, TPX], f16, tag="g16")
                sqvs, lnas, lnbs, dds, ees = [], [], [], [], []
                for h in range(2):
                    c0 = gi * TPX + h * HP
                    sqv = chpool.tile([128, HP], f32, tag=f"sqvh{h}")
                    nc.vector.tensor_mul(
                        sqv[:], gp[:, h * HP : (h + 1) * HP], r4_t[:, c0 : c0 + HP]
                    )
                    sqvs.append(sqv)
                for h in range(2):
                    ln_a = chpool.tile([128, HP], f32, tag=f"lnah{h}")
                    nc.scalar.activation(ln_a[:], sqvs[h][:], AF.Ln, bias=eps_t[:, 0:1])
                    ln_b = chpool.tile([128, HP], f32, tag=f"lnbh{h}")
                    nc.scalar.activation(ln_b[:], sqvs[h][:], AF.Ln, bias=1.0)
                    lnas.append(ln_a)
                    lnbs.append(ln_b)
                for h in range(2):
                    dd = chpool.tile([128, HP], f32, tag=f"ddh{h}")
                    nc.vector.scalar_tensor_tensor(
                        dd[:], lnas[h][:], 0.5, lnbs[h][:], OP.mult, OP.subtract
                    )
                    dds.append(dd)
                for h in range(2):
                    ee = chpool.tile([128, HP], f32, tag=f"eeh{h}")
                    nc.scalar.activation(ee[:], dds[h][:], AF.Exp)
                    ees.append(ee)
                for h in range(2):
                    c0 = gi * TPX + h * HP
                    nc.vector.tensor_mul(
                        gt_[:, h * HP : (h + 1) * HP], ees[h][:], rr_t[:, c0 : c0 + HP]
                    )
                g_tiles[gi] = gt_

            def emit_phase2(gi, j):
                t = first_tile[gi] + j
                gb = shpool.tile([128, TPX], f16, tag="gbc")
                nc.vector.stream_shuffle(gb[:], g_tiles[gi][:], [j] * 32)
                v = vpool.tile([128, TPX], f16, tag="vout")
                s_sl = s_tiles[gi][:, j * TPX : (j + 1) * TPX]
                nc.vector.tensor_mul(v[:], s_sl, gb[:])
                nc.sync.dma_start(out_d[:, t * TPX : (t + 1) * TPX], v[:])

            def emit_phase2_last(gi, j):
                # The PE is idle once the convs are done, so the tail group's
                # G broadcast goes back to a sel-mask matmul (PSUM), keeping
                # Vector free for the chain + the final multiplies. Half-split
                # so the broadcast starts on the chain's first finished half.
                t = first_tile[gi] + j
                HP = TPX // 2
                gb = bps.tile([128, TPX], f32, tag="warm")
                v = vpool.tile([128, TPX], f16, tag="vout")
                s_sl = s_tiles[gi][:, j * TPX : (j + 1) * TPX]
                for h in range(2):
                    sl = slice(h * HP, (h + 1) * HP)
                    nc.tensor.matmul(
                        gb[:, sl],
                        sel_t[:, j * NOCH : (j + 1) * NOCH],
                        g_tiles[gi][:, sl],
                        start=True,
                        stop=True,
                    )
                    nc.vector.tensor_mul(v[:, sl], s_sl[:, sl], gb[:, sl])
                nc.sync.dma_start(out_d[:, t * TPX : (t + 1) * TPX], v[:])

            # Software-pipelined emission: the reduce for a tile is emitted one
            # conv-tile later (covers the ACT add+square latency), the chain +
            # its tiles' phase2 (all Vector-side) as soon as the group's last
            # reduce is out.
            from collections import deque

            pend_red = deque()      # (gi, j, sq_tile)
            tiles_done = 0
            for gi, gsz in enumerate(GROUPS):
                s_tiles[gi] = sgpool.tile(
                    [128, gsz * TPX], f16, tag="sgroup", name=f"sgroup{gi}"
                )
                gp_tiles[gi] = gps.tile(
                    [128, TPX], f32, tag="redps", name=f"redps{gi}"
                )
                for j in range(gsz):
                    sq_j, last_mm = emit_conv_tile(gi, j)
                    tiles_done += 1
                    if tiles_done == 3:
                        # r constants: SWDGE transfer held behind tile 2 so it
                        # never competes with the startup image/weight DMAs;
                        # first consumer (chain of group 0) is ~15us later.
                        rr_dma = nc.gpsimd.dma_start(rr_t[:], rr_d[:])
                        add_dep_helper(
                            rr_dma.ins, last_mm.ins, sync=True, reason="late rr"
                        )
                        # r^2 per group; on Vector, which is otherwise idle
                        # until the first chain
                        for g2 in range(NGR):
                            nc.vector.tensor_mul(
                                r4_t[:, g2 * TPX : (g2 + 1) * TPX],
                                rr_t[:, g2 * TPX : (g2 + 1) * TPX],
                                rr_t[:, g2 * TPX : (g2 + 1) * TPX],
                            )
                    if pend_red:
                        rgi, rj, rsq = pend_red.popleft()
                        emit_red(rgi, rj, rsq, last_mm)
                        if rj == GROUPS[rgi] - 1:
                            emit_chain(rgi)
                            for k in range(GROUPS[rgi]):
                                emit_phase2(rgi, k)
                    pend_red.append((gi, j, sq_j))
            # drain: only the last group's reduce/chain/phase2 remain
            prev_mm = last_mm
            while pend_red:
                rgi, rj, rsq = pend_red.popleft()
                prev_mm = emit_red(rgi, rj, rsq, prev_mm)
                if rj == GROUPS[rgi] - 1:
                    if rgi == NGR - 1:
                        emit_chain_last(rgi)
                        for k in range(GROUPS[rgi]):
                            emit_phase2_last(rgi, k)
                    else:
                        emit_chain(rgi)
                        for k in range(GROUPS[rgi]):
                            emit_phase2(rgi, k)

    nc.compile()
    _BUILD_CACHE["nc"] = nc
    return nc


def _host_prep(u, Wf, bias):
    """Per-core input arrays. u [4,4,16,128,128], Wf [4,256,16,5,5], bias [4,256]."""
    u = np.ascontiguousarray(u, dtype=np.float32)
    Wf = np.ascontiguousarray(Wf, dtype=np.float32)
    bias = np.ascontiguousarray(bias, dtype=np.float32)

    # r(h,w) = 1/(8*nvalid); nvalid = clipped 5x5 window size
    nv = np.minimum(np.arange(H) + 2, H - 1) - np.maximum(np.arange(H) - 2, 0) + 1
    nvalid = np.outer(nv, nv).astype(np.float64)
    r = (1.0 / (8.0 * nvalid)).astype(np.float32)          # [H, W]

    # RR[p, gi*TPX + n] = r at pixel n of tile (first_tile[gi] + p%32 mod gsz)
    rr = np.zeros((128, NGR * TPX), np.float32)
    rflat = r.reshape(H * W)
    ft = 0
    for gi, gsz in enumerate(GROUPS):
        for p in range(128):
            t = ft + (p % 32) % gsz
            rr[p, gi * TPX : (gi + 1) * TPX] = rflat[t * TPX : (t + 1) * TPX]
        ft += gsz

    # reduce mask: MRED[p, j*128 + m] = 1 iff m == (p//32)*32 + j, so the
    # per-tile sq-sum for capsule oc lands on partition oc*32 + j and the
    # G broadcast is a quadrant-local stream_shuffle.
    mred = np.zeros((128, 8 * NOCH + 2), np.float16)
    for j in range(8):
        for p in range(128):
            mred[p, j * NOCH + (p // 32) * 32 + j] = 1.0

    # tail-group broadcast mask: SEL[p, j*128+m] = 1 iff p == (m//32)*32 + j
    sel = np.zeros((128, GROUPS[-1] * NOCH), np.float16)
    for j in range(GROUPS[-1]):
        for m in range(NOCH):
            sel[(m // 32) * 32 + j, j * NOCH + m] = 1.0

    bias_sum = bias.sum(axis=0)                            # [256]

    # weights: WT[p, ti*128 + o]; p = hlf*64 + ic*16 + cid
    # ti 0..9  (row pairs): ti = dyp*5+kw, dy = 2*dyp + hlf, taps (dy, kw)
    # ti 10..12 (col pairs): kw = 2*kwp + hlf, taps (4, kw); kw=5 -> 0
    wts = []
    for half in range(2):
        wt = np.zeros((128, NKT * NOCH), np.float16)
        Wh = Wf[:, half * NOCH : (half + 1) * NOCH]        # [4, 128, 16, 5, 5]
        for dyp in range(2):
            for kw in range(KS):
                ti = dyp * 5 + kw
                for hlf in range(2):
                    dy = 2 * dyp + hlf
                    # [4,128,16] -> [4,16,128] -> [64,128]
                    blk = Wh[:, :, :, dy, kw].transpose(0, 2, 1).reshape(64, NOCH)
                    wt[hlf * 64 : (hlf + 1) * 64, ti * NOCH : (ti + 1) * NOCH] = blk
        for kwp in range(3):
            ti = 10 + kwp
            for hlf in range(2):
                kw = 2 * kwp + hlf
                if kw >= KS:
                    continue
                blk = Wh[:, :, :, 4, kw].transpose(0, 2, 1).reshape(64, NOCH)
                wt[hlf * 64 : (hlf + 1) * 64, ti * NOCH : (ti + 1) * NOCH] = blk
        wts.append(wt)

    # padded image per batch: pad[c, yy, xx]; upad second copy shifted +1 row,
    # upad2 = rows 4..131 of pad (first half) and of the +1-col shift (second)
    upads = []
    upad2s = []
    for b in range(B):
        pad = np.zeros((CC, PADH, PADW), np.float16)
        pad[:, 2 : 2 + H, 2 : 2 + W] = u[b].reshape(CC, H, W)
        up = np.empty((128, PADH * PADW), np.float16)
        up[0:64] = pad.reshape(CC, -1)
        sh = np.zeros_like(pad)
        sh[:, 0 : PADH - 1] = pad[:, 1:PADH]
        up[64:128] = sh.reshape(CC, -1)
        upads.append(up)
        up2 = np.empty((128, PADH2 * PADW), np.float16)
        up2[0:64] = pad[:, 4 : 4 + PADH2].reshape(CC, -1)
        padx = np.zeros((CC, PADH2, PADW), np.float16)
        padx[:, :, 0 : PADW - 1] = pad[:, 4 : 4 + PADH2, 1:PADW]
        up2[64:128] = padx.reshape(CC, -1)
        upad2s.append(up2)

    in_maps = []
    for c in range(8):
        b, half = c // 2, c % 2
        mredb = mred.copy()
        bb = np.ascontiguousarray(
            bias_sum[half * NOCH : (half + 1) * NOCH], dtype=np.float32
        )
        mredb[:, 8 * NOCH : 8 * NOCH + 2] = bb.view(np.float16).reshape(128, 2)
        in_maps.append(
            {
                "upad": upads[b],
                "upad2": upad2s[b],
                "wt": wts[half],
                "mred": mredb,
                "sel": sel,
                "rr": rr,
            }
        )
    return in_maps


def _gather(results):
    out = np.empty((B, OC, OD, H, W), np.float32)
    for c in range(8):
        b, half = c // 2, c % 2
        o = results[c]["out"]                              # [128, NPIX] f16
        out[b, half * 4 : (half + 1) * 4] = o.reshape(4, OD, H, W)
    return out


def run(u, W, bias, trace=False):
    _ensure_path()
    from concourse.bass_utils import run_bass_kernel_spmd

    nc = _build_program()
    in_maps = _host_prep(u, W, bias)
    res = run_bass_kernel_spmd(nc, in_maps, list(range(8)), trace=trace)
    return _gather(res.results), res


def kernel(u, W, bias):
    out, _ = run(u, W, bias, trace=False)
    return out


# revision 19
# speedup vs baseline: 1.0580x; 1.0580x over previous
"""CapsuleLayer kernel for 8x TRN2 NeuronCores (Bass/Tile, SPMD).

Math (reference collapses because routing logits b stay zero):
  s[b,o,h,w]  = sum_ic conv2d(u[b,ic], W[ic], SAME) + sum_ic bias[ic]
              = conv2d(u[b] as 64ch, Wcat[256,64,5,5]) + bias_sum      (one conv)
  r(h,w)      = 1 / (8 * nvalid(h,w))              (input-independent constant)
  p           = r * s ; sq[oc] = sum_od p^2
  v           = p * sq/((1+sq)*sqrt(sq+1e-9))
  out[b,oc,od,h,w] = v

Sharding: 8 cores = (batch b in 0..4) x (OC half in 0..2). Each core runs a
64->128-channel 5x5 conv over one 128x128 image + squash, fully on-chip.

Conv as 13 matmuls per 512-pixel tile (K=128, M=128, N=512, fp16):
  - 10 row-paired: K packs 64ch x 2 kernel rows (dy in {0,1} / {2,3}) via a
    second image copy shifted one row (partitions 64-127), x 5 kw taps.
  - 3 col-paired: kernel row dy=4, K packs 64ch x 2 kw taps via a second
    tensor whose partitions 64-127 hold the image shifted one column.
  (25 taps * 64ch = 1600 = 12.5 * 128, so 13 matmuls is the K=128 floor.)

Squash: per-tile cross-partition sq-reduce via one PE matmul with a 0/1 mask
whose output partition is m = oc*32 + j (oc = capsule quadrant, j = tile index
within the group), so the G broadcast is a single Vector STREAM_SHUFFLE
(mask=[j]*32 replicates partition j within each 32-partition quadrant) instead
of a PE matmul. The scalar chain uses only {square, ln, exp} (single forced
ACT table set):  G = r * exp(0.5*ln(sq+1e-9) - ln(1+sq));  v = s * G.

Scheduling: reduce lagged one conv tile behind its ACT square (add_dep_helper),
a few PE warmup matmuls during the input DMA to pre-arm the HAM clock gate,
inputs split across both HWDGE rings (+ SWDGE for the r constants), f16 output.
"""

import numpy as np


def _ensure_path():
    try:
        import concourse.bass  # noqa: F401
    except ImportError:
        import sys

        for p in ("/opt/trn_rl_repo", "/root/.axon_site/_ro/trn_rl_repo"):
            if p not in sys.path:
                sys.path.insert(0, p)
        import concourse.bass  # noqa: F401


B, IC, CIN, H, W = 4, 4, 16, 128, 128
KS, OC, OD = 5, 8, 32
CC = IC * CIN            # 64 contraction channels
NOCH = 128               # out channels per core (4 capsules x 32 dims)
PADH, PADW = H + 5, W + 4   # 133 x 132 (extra pad row for the shifted copy)
PADH2 = 128              # upad2 rows = pad rows 4..131 (dy=4 taps only)
NPIX = H * W
TPX = 512                # pixels per tile (4 rows)
NT = NPIX // TPX         # 32 tiles
GROUPS = [8, 6, 6, 4, 3, 2, 3]   # tiles per squash group; small groups are
NGR = len(GROUPS)                # spread so tail chains don't stack on Vector
NKT = 13                 # matmuls per conv tile: 10 row-paired + 3 col-paired
NWARM = 8

_BUILD_CACHE = {}


def _build_program():
    """Build the SPMD Bass program (same for every core)."""
    if "nc" in _BUILD_CACHE:
        return _BUILD_CACHE["nc"]
    _ensure_path()
    import concourse.bacc as bacc
    import concourse.mybir as mybir
    import concourse.tile as tile
    from concourse.tile import add_dep_helper

    f32 = mybir.dt.float32
    f16 = mybir.dt.float16
    AF = mybir.ActivationFunctionType
    OP = mybir.AluOpType

    # Square/Ln/Exp/Identity all live in the 'natural_log_exp_and_others' ACT
    # table set, but the default set picker uses a different home set per
    # function (2 table reloads x 1.3us per group). Restrict the choice so a
    # single table load covers the whole kernel.
    if not getattr(bacc, "_capsule_act_patch", False):
        _orig_tables = bacc.get_activation_tables

        def _one_set_tables(arch):
            t = _orig_tables(arch)
            keep = "natural_log_exp_and_others"
            if keep in t:
                t = {k: (v if k == keep else set()) for k, v in t.items()}
            return t

        bacc.get_activation_tables = _one_set_tables
        bacc._capsule_act_patch = True

    nc = bacc.Bacc("TRN2", target_bir_lowering=False, debug=False, num_devices=8)

    upad_d = nc.dram_tensor("upad", [128, PADH * PADW], f16, kind="ExternalInput").ap()
    upad2_d = nc.dram_tensor("upad2", [128, PADH2 * PADW], f16, kind="ExternalInput").ap()
    wt_d = nc.dram_tensor("wt", [128, NKT * NOCH], f16, kind="ExternalInput").ap()
    # mred carries the f32 bias bit-pattern in 2 trailing f16 columns: a
    # standalone [128,1] f32 DMA degenerates to 128 4-byte packets that each
    # burn a round-robin slot on the ring (~3.4us of queue time).
    mred_d = nc.dram_tensor("mred", [128, 8 * NOCH + 2], f16, kind="ExternalInput").ap()
    sel_d = nc.dram_tensor("sel", [128, GROUPS[-1] * NOCH], f16, kind="ExternalInput").ap()
    rr_d = nc.dram_tensor("rr", [128, NGR * TPX], f32, kind="ExternalInput").ap()
    out_d = nc.dram_tensor("out", [128, NPIX], f16, kind="ExternalOutput").ap()

    with tile.TileContext(nc) as tc:
        with (
            tc.tile_pool(name="const", bufs=1) as cpool,
            tc.tile_pool(name="sg", bufs=3) as sgpool,
            tc.tile_pool(name="sq", bufs=4) as sqpool,
            tc.tile_pool(name="chain", bufs=2) as chpool,
            tc.tile_pool(name="gv", bufs=3) as gvpool,
            tc.tile_pool(name="sh", bufs=3) as shpool,
            tc.tile_pool(name="vout", bufs=3) as vpool,
            tc.tile_pool(name="cps", bufs=4, space="PSUM") as cps,
            tc.tile_pool(name="gps", bufs=2, space="PSUM") as gps,
            tc.tile_pool(name="bps", bufs=2, space="PSUM") as bps,
        ):
            # DMA order matters, and so do chunk sizes: the SDMA engines
            # round-robin between queues at PACKET granularity, so early
            # chunks are kept small and strictly in need-order; bulk data is
            # deferred. The r constants ride SWDGE, dep-anchored behind tile 2
            # so they stay out of the startup window entirely.
            wt_t = cpool.tile([128, NKT * NOCH], f16)
            nc.sync.dma_start(wt_t[:], wt_d[:])
            upad2_t = cpool.tile([128, PADH2 * PADW], f16)
            upad23 = upad2_t[:].rearrange("p (y x) -> p y x", x=PADW)
            u2src3 = upad2_d.rearrange("p (y x) -> p y x", x=PADW)
            nc.sync.dma_start(upad23[:, 0:16, :], u2src3[:, 0:16, :])
            mred_t = cpool.tile([128, 8 * NOCH + 2], f16)
            nc.sync.dma_start(mred_t[:], mred_d[:])
            bias_t = mred_t[:, 8 * NOCH : 8 * NOCH + 2].bitcast(f32)
            for r0, r1 in [(16, 32), (32, 56), (56, 88), (88, PADH2)]:
                nc.sync.dma_start(upad23[:, r0:r1, :], u2src3[:, r0:r1, :])
            sel_t = cpool.tile([128, GROUPS[-1] * NOCH], f16)
            nc.sync.dma_start(sel_t[:], sel_d[:])

            upad_t = cpool.tile([128, PADH * PADW], f16)
            upad3 = upad_t[:].rearrange("p (y x) -> p y x", x=PADW)
            usrc3 = upad_d.rearrange("p (y x) -> p y x", x=PADW)
            for r0, r1 in [(0, 7), (7, 15), (15, 31), (31, 55), (55, 85), (85, PADH)]:
                nc.scalar.dma_start(upad3[:, r0:r1, :], usrc3[:, r0:r1, :])

            rr_t = cpool.tile([128, NGR * TPX], f32)
            r4_t = cpool.tile([128, NGR * TPX], f32)
            eps_t = cpool.tile([128, 1], f32)
            nc.vector.memset(eps_t[:], 1e-9)

            # PE warmup: junk matmuls while the input DMAs land, so the HAM
            # clock gate is ramping toward 8/8 (2.4 GHz) when real conv work
            # starts. Operands alias the const-0.0 tile (written in the Bass
            # preamble BEFORE the Tile entry barrier) bitcast to f16.
            warm_ps = bps.tile([128, TPX], f32, tag="warm", name="warmps")
            c16 = nc.const_aps.aps[(f32, 0.0)].bitcast(f16)
            wrhs = c16[:, 0:1].to_broadcast((128, TPX))
            wlhs = c16[:, 0:1].to_broadcast((128, 128))
            for k in range(NWARM):
                nc.tensor.matmul(
                    warm_ps[:],
                    wlhs,
                    wrhs,
                    start=(k == 0),
                    stop=(k == NWARM - 1),
                )

            first_tile = [0] * NGR      # first global tile index per group
            acc = 0
            for gi, gsz in enumerate(GROUPS):
                first_tile[gi] = acc
                acc += gsz

            s_tiles = {}
            g_tiles = {}
            gp_tiles = {}

            def emit_conv_tile(gi, j):
                t = first_tile[gi] + j
                y0 = 4 * t
                cp = cps.tile([128, TPX], f32, tag="convps")
                ti = 0
                last_mm = None
                # 10 row-paired matmuls (dy in {0,1} and {2,3})
                for dyp in range(2):
                    for kw in range(KS):
                        rhs = upad3[:, y0 + 2 * dyp : y0 + 2 * dyp + 4, kw : kw + W]
                        last_mm = nc.tensor.matmul(
                            cp[:],
                            wt_t[:, ti * NOCH : (ti + 1) * NOCH],
                            rhs,
                            start=(ti == 0),
                            stop=False,
                        )
                        ti += 1
                # 3 col-paired matmuls (dy=4, kw pairs {0,1},{2,3},{4,-})
                for kwp in range(3):
                    rhs = upad23[:, y0 : y0 + 4, 2 * kwp : 2 * kwp + W]
                    last_mm = nc.tensor.matmul(
                        cp[:],
                        wt_t[:, ti * NOCH : (ti + 1) * NOCH],
                        rhs,
                        start=False,
                        stop=(ti == NKT - 1),
                    )
                    ti += 1
                # Square(cp + bias) straight from PSUM (fused bias, faster
                # PSUM read, and independent of the add below)
                sq = sqpool.tile([128, TPX], f16, tag="sqt")
                nc.scalar.activation(sq[:], cp[:], AF.Square, bias=bias_t[:, 0:1])
                s_sl = s_tiles[gi][:, j * TPX : (j + 1) * TPX]
                nc.scalar.add(s_sl, cp[:], bias_t[:, 0:1])
                return sq, last_mm

            def emit_red(gi, j, sq, anchor):
                gsz = GROUPS[gi]
                red_mm = nc.tensor.matmul(
                    gp_tiles[gi][:],
                    mred_t[:, j * NOCH : (j + 1) * NOCH],
                    sq[:],
                    start=(j == 0),
                    stop=(j == gsz - 1),
                )
                if anchor is not None:
                    # keep the reduce AFTER the just-emitted conv tile in the
                    # PE stream so its ACT square input is long done (the
                    # scheduler would otherwise hoist it next to its producer
                    # and stall the PE ~0.5-1us per tile)
                    add_dep_helper(
                        red_mm.ins, anchor.ins, sync=True, reason="lag red"
                    )
                return red_mm

            def emit_chain(gi):
                gp = gp_tiles[gi]
                r4 = r4_t[:, gi * TPX : (gi + 1) * TPX]
                r1 = rr_t[:, gi * TPX : (gi + 1) * TPX]
                sqv = chpool.tile([128, TPX], f32, tag="sqv")
                nc.vector.tensor_mul(sqv[:], gp[:], r4)
                ln_a = chpool.tile([128, TPX], f32, tag="ln_a")
                nc.scalar.activation(ln_a[:], sqv[:], AF.Ln, bias=eps_t[:, 0:1])
                ln_b = chpool.tile([128, TPX], f32, tag="ln_b")
                nc.scalar.activation(ln_b[:], sqv[:], AF.Ln, bias=1.0)
                dd = chpool.tile([128, TPX], f32, tag="dd")
                nc.vector.scalar_tensor_tensor(
                    dd[:], ln_a[:], 0.5, ln_b[:], OP.mult, OP.subtract
                )
                ee = chpool.tile([128, TPX], f32, tag="ee")
                nc.scalar.activation(ee[:], dd[:], AF.Exp)
                gt_ = gvpool.tile([128, TPX], f16, tag="g16")
                nc.vector.tensor_mul(gt_[:], ee[:], r1)
                g_tiles[gi] = gt_

            def emit_chain_last(gi):
                # Half-split chain for the tail group: the Scalar/Vector stages
                # of the two halves ping-pong, cutting the serial latency that
                # sits after the final conv matmul.
                gp = gp_tiles[gi]
                HP = TPX // 2
                gt_ = gvpool.tile([128, TPX], f16, tag="g16")
                sqvs, lnas, lnbs, dds, ees = [], [], [], [], []
                for h in range(2):
                    c0 = gi * TPX + h * HP
                    sqv = chpool.tile([128, HP], f32, tag=f"sqvh{h}")
                    nc.vector.tensor_mul(
                        sqv[:], gp[:, h * HP : (h + 1) * HP], r4_t[:, c0 : c0 + HP]
                    )
                    sqvs.append(sqv)
                for h in range(2):
                    ln_a = chpool.tile([128, HP], f32, tag=f"lnah{h}")
                    nc.scalar.activation(ln_a[:], sqvs[h][:], AF.Ln, bias=eps_t[:, 0:1])
                    ln_b = chpool.tile([128, HP], f32, tag=f"lnbh{h}")
                    nc.scalar.activation(ln_b[:], sqvs[h][:], AF.Ln, bias=1.0)
                    lnas.append(ln_a)
                    lnbs.append(ln_b)
                for h in range(2):
                    dd = chpool.tile([128, HP], f32, tag=f"ddh{h}")
                    nc.vector.scalar_tensor_tensor(
                        dd[:], lnas[h][:], 0.5, lnbs[h][:], OP.mult, OP.subtract
                    )
                    dds.append(dd)
                for h in range(2):
                    ee = chpool.tile([128, HP], f32, tag=f"eeh{h}")
                    nc.scalar.activation(ee[:], dds[h][:], AF.Exp)
                    ees.append(ee)
                for h in range(2):
                    c0 = gi * TPX + h * HP
                    nc.vector.tensor_mul(
                        gt_[:, h * HP : (h + 1) * HP], ees[h][:], rr_t[:, c0 : c0 + HP]
                    )
                g_tiles[gi] = gt_

            def emit_phase2(gi, j):
                t = first_tile[gi] + j
                gb = shpool.tile([128, TPX], f16, tag="gbc")
                # shuffle permutes partitions; an f32 view halves the element
                # count for the same bytes (DVE is element-rate-bound)
                nc.vector.stream_shuffle(
                    gb[:].bitcast(f32), g_tiles[gi][:].bitcast(f32), [j] * 32
                )
                v = vpool.tile([128, TPX], f16, tag="vout")
                s_sl = s_tiles[gi][:, j * TPX : (j + 1) * TPX]
                # late groups: the multiply rides GpSimd so Vector stays clear
                # for the tail chains
                eng = nc.gpsimd if gi >= NGR - 3 else nc.vector
                eng.tensor_mul(v[:], s_sl, gb[:])
                nc.sync.dma_start(out_d[:, t * TPX : (t + 1) * TPX], v[:])

            def emit_phase2_last(gi, j):
                # The PE is idle once the convs are done, so the tail group's
                # G broadcast goes back to a sel-mask matmul (PSUM), keeping
                # Vector free for the chain + the final multiplies. Half-split
                # so the broadcast starts on the chain's first finished half.
                t = first_tile[gi] + j
                HP = TPX // 2
                gb = bps.tile([128, TPX], f32, tag="warm")
                v = vpool.tile([128, TPX], f16, tag="vout")
                s_sl = s_tiles[gi][:, j * TPX : (j + 1) * TPX]
                for h in range(2):
                    sl = slice(h * HP, (h + 1) * HP)
                    nc.tensor.matmul(
                        gb[:, sl],
                        sel_t[:, j * NOCH : (j + 1) * NOCH],
                        g_tiles[gi][:, sl],
                        start=True,
                        stop=True,
                    )
                    nc.vector.tensor_mul(v[:, sl], s_sl[:, sl], gb[:, sl])
                nc.sync.dma_start(out_d[:, t * TPX : (t + 1) * TPX], v[:])

            # Software-pipelined emission: the reduce for a tile is emitted one
            # conv-tile later (covers the ACT add+square latency), the chain +
            # its tiles' phase2 (all Vector-side) as soon as the group's last
            # reduce is out.
            from collections import deque

            pend_red = deque()      # (gi, j, sq_tile)
            tiles_done = 0
            for gi, gsz in enumerate(GROUPS):
                s_tiles[gi] = sgpool.tile(
                    [128, gsz * TPX], f16, tag="sgroup", name=f"sgroup{gi}"
                )
                gp_tiles[gi] = gps.tile(
                    [128, TPX], f32, tag="redps", name=f"redps{gi}"
                )
                for j in range(gsz):
                    sq_j, last_mm = emit_conv_tile(gi, j)
                    tiles_done += 1
                    if tiles_done == 1:
                        # r constants: SWDGE transfer held behind tile 2 so it
                        # never competes with the startup image/weight DMAs;
                        # first consumer (chain of group 0) is ~15us later.
                        rr_dma = nc.gpsimd.dma_start(rr_t[:], rr_d[:])
                        add_dep_helper(
                            rr_dma.ins, last_mm.ins, sync=True, reason="late rr"
                        )
                        # r^2 per group; on Vector, which is otherwise idle
                        # until the first chain
                        for g2 in range(NGR):
                            nc.vector.tensor_mul(
                                r4_t[:, g2 * TPX : (g2 + 1) * TPX],
                                rr_t[:, g2 * TPX : (g2 + 1) * TPX],
                                rr_t[:, g2 * TPX : (g2 + 1) * TPX],
                            )
                    if pend_red:
                        rgi, rj, rsq = pend_red.popleft()
                        emit_red(rgi, rj, rsq, last_mm)
                        if rj == GROUPS[rgi] - 1:
                            emit_chain(rgi)
                            for k in range(GROUPS[rgi]):
                                emit_phase2(rgi, k)
                    pend_red.append((gi, j, sq_j))
            # drain: only the last group's reduce/chain/phase2 remain
            prev_mm = last_mm
            while pend_red:
                rgi, rj, rsq = pend_red.popleft()
                prev_mm = emit_red(rgi, rj, rsq, prev_mm)
                if rj == GROUPS[rgi] - 1:
                    if rgi == NGR - 1:
                        emit_chain_last(rgi)
                        for k in range(GROUPS[rgi]):
                            emit_phase2_last(rgi, k)
                    else:
                        emit_chain(rgi)
                        for k in range(GROUPS[rgi]):
                            emit_phase2(rgi, k)

    nc.compile()
    _BUILD_CACHE["nc"] = nc
    return nc


def _host_prep(u, Wf, bias):
    """Per-core input arrays. u [4,4,16,128,128], Wf [4,256,16,5,5], bias [4,256]."""
    u = np.ascontiguousarray(u, dtype=np.float32)
    Wf = np.ascontiguousarray(Wf, dtype=np.float32)
    bias = np.ascontiguousarray(bias, dtype=np.float32)

    # r(h,w) = 1/(8*nvalid); nvalid = clipped 5x5 window size
    nv = np.minimum(np.arange(H) + 2, H - 1) - np.maximum(np.arange(H) - 2, 0) + 1
    nvalid = np.outer(nv, nv).astype(np.float64)
    r = (1.0 / (8.0 * nvalid)).astype(np.float32)          # [H, W]

    # RR[p, gi*TPX + n] = r at pixel n of tile (first_tile[gi] + p%32 mod gsz)
    rr = np.zeros((128, NGR * TPX), np.float32)
    rflat = r.reshape(H * W)
    ft = 0
    for gi, gsz in enumerate(GROUPS):
        for p in range(128):
            t = ft + (p % 32) % gsz
            rr[p, gi * TPX : (gi + 1) * TPX] = rflat[t * TPX : (t + 1) * TPX]
        ft += gsz

    # reduce mask: MRED[p, j*128 + m] = 1 iff m == (p//32)*32 + j, so the
    # per-tile sq-sum for capsule oc lands on partition oc*32 + j and the
    # G broadcast is a quadrant-local stream_shuffle.
    mred = np.zeros((128, 8 * NOCH + 2), np.float16)
    for j in range(8):
        for p in range(128):
            mred[p, j * NOCH + (p // 32) * 32 + j] = 1.0

    # tail-group broadcast mask: SEL[p, j*128+m] = 1 iff p == (m//32)*32 + j
    sel = np.zeros((128, GROUPS[-1] * NOCH), np.float16)
    for j in range(GROUPS[-1]):
        for m in range(NOCH):
            sel[(m // 32) * 32 + j, j * NOCH + m] = 1.0

    bias_sum = bias.sum(axis=0)                            # [256]

    # weights: WT[p, ti*128 + o]; p = hlf*64 + ic*16 + cid
    # ti 0..9  (row pairs): ti = dyp*5+kw, dy = 2*dyp + hlf, taps (dy, kw)
    # ti 10..12 (col pairs): kw = 2*kwp + hlf, taps (4, kw); kw=5 -> 0
    wts = []
    for half in range(2):
        wt = np.zeros((128, NKT * NOCH), np.float16)
        Wh = Wf[:, half * NOCH : (half + 1) * NOCH]        # [4, 128, 16, 5, 5]
        for dyp in range(2):
            for kw in range(KS):
                ti = dyp * 5 + kw
                for hlf in range(2):
                    dy = 2 * dyp + hlf
                    # [4,128,16] -> [4,16,128] -> [64,128]
                    blk = Wh[:, :, :, dy, kw].transpose(0, 2, 1).reshape(64, NOCH)
                    wt[hlf * 64 : (hlf + 1) * 64, ti * NOCH : (ti + 1) * NOCH] = blk
        for kwp in range(3):
            ti = 10 + kwp
            for hlf in range(2):
                kw = 2 * kwp + hlf
                if kw >= KS:
                    continue
                blk = Wh[:, :, :, 4, kw].transpose(0, 2, 1).reshape(64, NOCH)
                wt[hlf * 64 : (hlf + 1) * 64, ti * NOCH : (ti + 1) * NOCH] = blk
        wts.append(wt)

    # padded image per batch: pad[c, yy, xx]; upad second copy shifted +1 row,
    # upad2 = rows 4..131 of pad (first half) and of the +1-col shift (second)
    upads = []
    upad2s = []
    for b in range(B):
        pad = np.zeros((CC, PADH, PADW), np.float16)
        pad[:, 2 : 2 + H, 2 : 2 + W] = u[b].reshape(CC, H, W)
        up = np.empty((128, PADH * PADW), np.float16)
        up[0:64] = pad.reshape(CC, -1)
        sh = np.zeros_like(pad)
        sh[:, 0 : PADH - 1] = pad[:, 1:PADH]
        up[64:128] = sh.reshape(CC, -1)
        upads.append(up)
        up2 = np.empty((128, PADH2 * PADW), np.float16)
        up2[0:64] = pad[:, 4 : 4 + PADH2].reshape(CC, -1)
        padx = np.zeros((CC, PADH2, PADW), np.float16)
        padx[:, :, 0 : PADW - 1] = pad[:, 4 : 4 + PADH2, 1:PADW]
        up2[64:128] = padx.reshape(CC, -1)
        upad2s.append(up2)

    in_maps = []
    for c in range(8):
        b, half = c // 2, c % 2
        mredb = mred.copy()
        bb = np.ascontiguousarray(
            bias_sum[half * NOCH : (half + 1) * NOCH], dtype=np.float32
        )
        mredb[:, 8 * NOCH : 8 * NOCH + 2] = bb.view(np.float16).reshape(128, 2)
        in_maps.append(
            {
                "upad": upads[b],
                "upad2": upad2s[b],
                "wt": wts[half],
                "mred": mredb,
                "sel": sel,
                "rr": rr,
            }
        )
    return in_maps


def _gather(results):
    out = np.empty((B, OC, OD, H, W), np.float32)
    for c in range(8):
        b, half = c // 2, c % 2
        o = results[c]["out"]                              # [128, NPIX] f16
        out[b, half * 4 : (half + 1) * 4] = o.reshape(4, OD, H, W)
    return out


def run(u, W, bias, trace=False):
    _ensure_path()
    from concourse.bass_utils import run_bass_kernel_spmd

    nc = _build_program()
    in_maps = _host_prep(u, W, bias)
    res = run_bass_kernel_spmd(nc, in_maps, list(range(8)), trace=trace)
    return _gather(res.results), res


def kernel(u, W, bias):
    out, _ = run(u, W, bias, trace=False)
    return out
